# revision 54
# baseline (speedup 1.0000x reference)
"""Pre-LN transformer block (B=2,T=2048,C=1024,H=16) on 8 TRN2 NeuronCores.

Two SPMD launches:
  L1: tensor-parallel over heads (2 heads/core) - LN1 stats on-chip
      (replicated, folded algebraically into the QKV matmuls), causal
      attention with on-chip softmax, normalized attn^T output per core.
  L2: parallel over token rows (512 rows/core) - output projection +
      residual, LN2, FFN (relu) + residual.
Host work between launches is just resharding: slicing/concat and dtype
casts. All matmuls run in bf16 with fp32 PSUM accumulation; residuals
are carried in fp32.
"""
"""Transformer block on 8 TRN2 cores: L1 = head-parallel attention, L2 = row-parallel proj+FFN."""
import contextlib
import numpy as np
import ml_dtypes

import concourse.bass as bass
import concourse.mybir as mybir
import concourse.tile as tile
from concourse import bacc
from concourse.masks import make_identity

bf16 = ml_dtypes.bfloat16
FP32 = mybir.dt.float32
BF16 = mybir.dt.bfloat16
AF = mybir.ActivationFunctionType

B, T, C, H = 2, 2048, 1024, 16
HS = C // H          # 64
NCORES = 8
HPC = H // NCORES    # 2 heads per core
TOK = B * T          # 4096
EPS = 1e-5
CT = C // 128        # 8 c-tiles
NCH = TOK // 512     # 8 512-col chunks of token axis
QB = 512             # query block
ROWS = TOK // NCORES # 512 rows per core in L2
HID = 4 * C          # 4096
HT = HID // 128      # 32 hidden tiles
MT = ROWS // 128     # 4 token tiles in L2


def build_l1(debug=False, use_beta=True):
    """Head-parallel attention v2.

    Scores run row-tiled (both heads concurrently on disjoint PE row
    groups), exp drains head-paired [128,1024] PSUM tiles in one ACT op
    writing fp8 probs, attn@V runs fp8 DoubleRow over 256-key spans with
    the softmax denominator riding as a 65th V column. No on-chip
    normalization: the host divides by the denominator row. PSUM drains
    ride on DVE; squares for LN1 stats are computed on GPSIMD (halves
    the input DMA).
    """
    nc = bacc.Bacc("TRN2", target_bir_lowering=False, debug=False, num_devices=NCORES)
    CP = CT // 2
    F8 = mybir.dt.float8e4
    xt_d = nc.dram_tensor("xt", [NCH, 128, CP, 2, 512], F8, kind="ExternalInput").ap()
    sq_d = nc.dram_tensor("sq8", [NCH, 128, CP, 2, 512], F8, kind="ExternalInput").ap()
    wq_d = nc.dram_tensor("wq", [128, CP, 2, 128], F8, kind="ExternalInput").ap()
    wk_d = nc.dram_tensor("wk", [128, CP, 2, 128], F8, kind="ExternalInput").ap()
    wv_d = nc.dram_tensor("wv", [128, CP, 2, 128], F8, kind="ExternalInput").ap()
    # negated column sums of wq/wk/wv and W.T @ beta1, all [128,1] fp32
    nws_d = nc.dram_tensor("nws", [128, 3], FP32, kind="ExternalInput").ap()
    wb_d = nc.dram_tensor("wb", [128, 3], FP32, kind="ExternalInput").ap()
    tri_d = nc.dram_tensor("tri2", [128, 2, 128], F8, kind="ExternalInput").ap()
    # per (head, qblock): rows hl*65..hl*65+63 = unnormalized attn,
    # row hl*65+64 = softmax denominator
    out_d = nc.dram_tensor("attn_out", [130, TOK], BF16, kind="ExternalOutput").ap()

    with tile.TileContext(nc) as tc, contextlib.ExitStack() as ctx:
        consts = ctx.enter_context(tc.tile_pool(name="consts", bufs=1))
        hpool = ctx.enter_context(tc.tile_pool(name="hT", bufs=1))
        stats = ctx.enter_context(tc.tile_pool(name="stats", bufs=1))
        tmp = ctx.enter_context(tc.tile_pool(name="tmp", bufs=3))
        qkv = ctx.enter_context(tc.tile_pool(name="qkv", bufs=1))
        probs_p = ctx.enter_context(tc.tile_pool(name="probs", bufs=12))
        attn_sb_p = ctx.enter_context(tc.tile_pool(name="attn_sb", bufs=3))
        # PSUM: 2x [128,1024] pair tiles (4 banks; scores + stats + qkv +
        # transposes all rotate here) + 4x [128,512] acc banks: dedicated
        # attention accumulators per (batch, head).
        ps_pair = ctx.enter_context(tc.tile_pool(name="ps_pair", bufs=2, space="PSUM"))
        ps_acc = ctx.enter_context(tc.tile_pool(name="ps_acc", bufs=1, space="PSUM"))

        def acc_tile(i):
            return ps_acc.tile([128, 512], FP32, tag=f"acc{i}", name=f"acc{i}")

        # ---- constants ----
        ones_sb = consts.tile([128, 128], BF16)
        nc.vector.memset(ones_sb, 1.0)
        eps_sb = consts.tile([128, 1], FP32)
        nc.vector.memset(eps_sb, EPS)
        ident = consts.tile([128, 128], BF16)
        make_identity(nc, ident)

        # PE warm-up spin: keep HAM busy while input DMAs stream
        warm_ps = ps_pair.tile([128, 1024], FP32, tag="sc")
        for _ in range(22):
            nc.tensor.matmul(warm_ps[:, 0:128], ones_sb, ones_sb[:, 0:128], start=True, stop=True)

        tri_sb = consts.tile([128, 2, 128], F8)
        nc.sync.dma_start(out=tri_sb, in_=tri_d)
        wq_sb = consts.tile([128, CP, 2, 128], F8)
        nc.sync.dma_start(out=wq_sb, in_=wq_d)
        wk_sb = consts.tile([128, CP, 2, 128], F8)
        nc.scalar.dma_start(out=wk_sb, in_=wk_d)
        wv_sb = consts.tile([128, CP, 2, 128], F8)
        nc.gpsimd.dma_start(out=wv_sb, in_=wv_d)
        nws_sb = consts.tile([128, 3], FP32)
        nc.scalar.dma_start(out=nws_sb, in_=nws_d)
        wb_sb = consts.tile([128, 3], FP32)
        nc.scalar.dma_start(out=wb_sb, in_=wb_d)
        ones8 = consts.tile([128, 2, 128], F8)
        nc.vector.memset(ones8, 1.0)
        lnsc_sb = consts.tile([128, 1], FP32)
        nc.vector.memset(lnsc_sb, -2.7725887)

        # ---- load xT + squares: each chunk is STRIPED across all three
        # DMA queues so chunks land in order at aggregate HBM bandwidth
        # (a whole-tensor round-robin makes chunk c wait ~3 queue-depths,
        # and stats MMs gated on late chunks stall the in-order PE queue) ----
        xqs = [None] * NCH
        sqs = [None] * NCH
        engs = (nc.sync, nc.scalar, nc.gpsimd)
        qi = 0
        for j in list(range(NCH)):
            xq_t = hpool.tile([128, CP, 2, 512], F8, tag=f"xq{j}", name=f"xq{j}")
            sq_t = hpool.tile([128, CP, 2, 512], F8, tag=f"sq{j}", name=f"sq{j}")
            for t_sb, t_d in ((xq_t, xt_d[j]), (sq_t, sq_d[j])):
                for h in range(2):
                    engs[qi % 3].dma_start(out=t_sb[:, 2 * h:2 * h + 2],
                                           in_=t_d[:, 2 * h:2 * h + 2])
                    qi += 1
            xqs[j] = xq_t
            sqs[j] = sq_t

        # ---- LN1 stats, computed per chunk-pair so the pipeline can
        # start before the whole input has streamed in. ones stationary
        # makes every PSUM partition carry the column sums -> stats born
        # broadcast across partitions. ----
        rstd_b = stats.tile([128, TOK], BF16, tag="rstd_b")
        murstd_b = stats.tile([128, TOK], BF16, tag="murstd_b")
        mu_b = stats.tile([128, TOK], FP32, tag="mu_b")
        var_b = stats.tile([128, TOK], FP32, tag="var_b")

        def stats_chunk(j):
            sl = slice(j * 512, (j + 1) * 512)
            ps_st = ps_pair.tile([128, 1024], FP32, tag="sc")
            ps_sum = ps_st[:, 0:512]
            ps_sq = ps_st[:, 512:1024]
            for cp in range(CP):
                nc.tensor.matmul(ps_sum, ones8, xqs[j][:, cp],
                                 start=(cp == 0), stop=(cp == CP - 1),
                                 skip_group_check=True,
                                 perf_mode=mybir.MatmulPerfMode.DoubleRow)
                nc.tensor.matmul(ps_sq, ones8, sqs[j][:, cp],
                                 start=(cp == 0), stop=(cp == CP - 1),
                                 skip_group_check=True,
                                 perf_mode=mybir.MatmulPerfMode.DoubleRow)
            nc.vector.tensor_scalar_mul(mu_b[:, sl], ps_sum, 1.0 / C)
            nc.vector.tensor_scalar_mul(var_b[:, sl], ps_sq, 1.0 / C)
            mu2 = tmp.tile([128, 512], BF16, tag="mu2")
            nc.vector.tensor_mul(mu2, mu_b[:, sl], mu_b[:, sl])
            nc.vector.tensor_sub(var_b[:, sl], var_b[:, sl], mu2)

        def rstd_span(lo, hi):
            # rstd = (var+eps)^-0.5 = Exp(-0.5*Ln(var+eps))
            # (same natural_log_exp table set as the softmax exps)
            sl = slice(lo, hi)
            nc.scalar.activation(out=var_b[:, sl], in_=var_b[:, sl], func=AF.Ln,
                                 bias=eps_sb, scale=1.0)
            nc.scalar.activation(out=rstd_b[:, sl], in_=var_b[:, sl], func=AF.Exp,
                                 scale=-0.5, bias=lnsc_sb)
            nc.vector.tensor_mul(murstd_b[:, sl], mu_b[:, sl], rstd_b[:, sl])

        def rstd_pair(p):
            rstd_span(p * 1024, (p + 1) * 1024)

        # ---- QT/KT/VT on RAW xT; LN folded in afterwards:
        #      QT = rstd*(Wq.T@xT) + (-colsum(Wq))*murstd + Wq.T@beta ----
        qt_sb = qkv.tile([128, TOK], BF16, tag="qt")
        kt_sb = qkv.tile([128, TOK], BF16, tag="kt")
        vt_sb = qkv.tile([128, TOK], BF16, tag="vt")
        wtriples = ((wv_sb, 2, vt_sb), (wq_sb, 0, qt_sb), (wk_sb, 1, kt_sb))

        def qkv_fold(tsb, idx, sl, ps):
            # drain + rstd scale fused on DVE (PSUM read)
            nc.vector.tensor_mul(tsb[:, sl], ps, rstd_b[:, sl])
            nc.vector.scalar_tensor_tensor(
                tsb[:, sl], murstd_b[:, sl], nws_sb[:, idx:idx + 1], tsb[:, sl],
                op0=mybir.AluOpType.mult, op1=mybir.AluOpType.add)
            if use_beta:
                nc.vector.tensor_scalar_add(tsb[:, sl], tsb[:, sl],
                                            wb_sb[:, idx:idx + 1])

        # V' pair tiles for fp8 DoubleRow attn@V: [128 tok-in-pair, 2 slot,
        # 144]: cols hl*65..hl*65+63 = V feats, col hl*65+64 = ones (denom),
        # cols 130..143 pad so the DR slot step stays 16-aligned.
        def vt_build2(tp):
            # xbar DMA transpose: off the PE and PSUM entirely
            vtp = vtps[tp]
            for o in range(2):
                tt = 2 * tp + o
                vst = tmp.tile([128, 128], BF16, tag="vst")
                nc.sync.dma_start_transpose(out=vst,
                                            in_=vt_sb[:, tt * 128:(tt + 1) * 128])
                nc.vector.tensor_copy(
                    vtp[:, o, 0:130].rearrange("p (g c) -> p g c", g=2)[:, :, 0:64],
                    vst.rearrange("p (g c) -> p g c", g=2))
                nc.vector.memset(vtp[:, o, 64:65], 1.0)
                nc.vector.memset(vtp[:, o, 129:130], 1.0)

        vtps = []
        for tp in range(TOK // 256):
            vtp_tile = qkv.tile([128, 2, 144], F8, tag=f"v{tp}", name=f"v{tp}")
            vtps.append(vtp_tile)

        # ---- per chunk-pair pipeline: stats -> rstd -> QKV folds -> V'
        # tiles. Pair 0 unblocks batch-0 attention j=0/1 within ~10us of
        # kernel start; later pairs stream in under the attention. ----
        ready_pair = [False] * 4
        ready_j0 = [False]

        def stats_chunk_acc(j):
            # batch-1 stats during batch-0 attention: use the acc banks so
            # the score-tile rotation is never blocked
            sl = slice(j * 512, (j + 1) * 512)
            ps_sum = acc_tile(0)
            ps_sq = acc_tile(1)
            for cp in range(CP):
                nc.tensor.matmul(ps_sum, ones8, xqs[j][:, cp],
                                 start=(cp == 0), stop=(cp == CP - 1),
                                 skip_group_check=True,
                                 perf_mode=mybir.MatmulPerfMode.DoubleRow)
                nc.tensor.matmul(ps_sq, ones8, sqs[j][:, cp],
                                 start=(cp == 0), stop=(cp == CP - 1),
                                 skip_group_check=True,
                                 perf_mode=mybir.MatmulPerfMode.DoubleRow)
            nc.vector.tensor_scalar_mul(mu_b[:, sl], ps_sum, 1.0 / C)
            nc.vector.tensor_scalar_mul(var_b[:, sl], ps_sq, 1.0 / C)
            mu2 = tmp.tile([128, 512], BF16, tag="mu2")
            nc.vector.tensor_mul(mu2, mu_b[:, sl], mu_b[:, sl])
            nc.vector.tensor_sub(var_b[:, sl], var_b[:, sl], mu2)

        def pairs_gen():
            # chunk 0 processed alone: unblocks attention j=0 ~8us earlier
            # than waiting for the whole first chunk-pair
            for c in range(2):
                stats_chunk(c)
                yield
                rstd_span(c * 512, (c + 1) * 512)
                yield
                for wsb, idx, tsb in ((wk_sb, 1, kt_sb), (wq_sb, 0, qt_sb),
                                      (wv_sb, 2, vt_sb)):
                    ps_pr = ps_pair.tile([128, 1024], FP32, tag="sc")
                    for cp in range(CP):
                        nc.tensor.matmul(
                            ps_pr[:, 0:512], wsb[:, cp], xqs[c][:, cp],
                            start=(cp == 0), stop=(cp == CP - 1),
                            skip_group_check=True,
                            perf_mode=mybir.MatmulPerfMode.DoubleRow)
                        yield
                    qkv_fold(tsb, idx, slice(c * 512, (c + 1) * 512),
                             ps_pr[:, 0:512])
                    yield
                vt_build2(2 * c)
                vt_build2(2 * c + 1)
                yield
                if c == 0:
                    ready_j0[0] = True
            ready_pair[0] = True
            # pair 1: score-pool tiles (pair-granular, LDW-amortized)
            for p in (1,):
                stats_chunk(2 * p)
                yield
                stats_chunk(2 * p + 1)
                yield
                rstd_pair(p)
                yield
                sl = slice(p * 1024, (p + 1) * 1024)
                for wsb, idx, tsb in ((wk_sb, 1, kt_sb), (wq_sb, 0, qt_sb),
                                      (wv_sb, 2, vt_sb)):
                    ps_pr = ps_pair.tile([128, 1024], FP32, tag="sc")
                    for cp in range(CP):
                        for h in range(2):
                            nc.tensor.matmul(
                                ps_pr[:, h * 512:(h + 1) * 512],
                                wsb[:, cp], xqs[2 * p + h][:, cp],
                                start=(cp == 0), stop=(cp == CP - 1),
                                skip_group_check=True,
                                perf_mode=mybir.MatmulPerfMode.DoubleRow)
                        yield
                    qkv_fold(tsb, idx, sl, ps_pr)
                    yield
                for tp in range(4 * p, 4 * p + 4):
                    vt_build2(tp)
                    yield
                ready_pair[p] = True
            # batch-1 pairs under batch-0 attention: acc0/1 banks only
            for p in range(2, 4):
                stats_chunk_acc(2 * p)
                yield
                stats_chunk_acc(2 * p + 1)
                yield
                rstd_pair(p)
                yield
                for wsb, idx, tsb in ((wk_sb, 1, kt_sb), (wq_sb, 0, qt_sb),
                                      (wv_sb, 2, vt_sb)):
                    for h in range(2):
                        sl = slice((2 * p + h) * 512, (2 * p + h + 1) * 512)
                        ps_h = acc_tile(h)
                        for cp in range(CP):
                            nc.tensor.matmul(
                                ps_h, wsb[:, cp], xqs[2 * p + h][:, cp],
                                start=(cp == 0), stop=(cp == CP - 1),
                                skip_group_check=True,
                                perf_mode=mybir.MatmulPerfMode.DoubleRow)
                            yield
                        qkv_fold(tsb, idx, sl, ps_h)
                    yield
                for tp in range(4 * p, 4 * p + 4):
                    vt_build2(tp)
                    yield
                ready_pair[p] = True

        # ---- attention: both heads together. Scores for head0/head1 run
        # row-tiled on disjoint PE row groups into the two halves of one
        # [128,1024] PSUM tile; one exp covers both heads; attn@V is fp8
        # DoubleRow over 256-key spans. ----
        scale = C ** -0.5

        # Per-batch dedicated accumulator banks (b0: acc2/3, b1: acc0/1)
        # so the two attention streams interleave freely at kp granularity
        # with no cross-batch rotation hazard on the PE queue.
        def attn_batch(b):
            acc_base = 2 - 2 * b
            for j in range(T // QB):
                # b1 waits for ALL pairs: its accumulator banks (acc0/1)
                # double as the batch-1 stats/qkv banks until then
                while not (ready_j0[0] if (b == 0 and j == 0)
                           else ready_pair[3 if b else j // 2]):
                    yield
                q0 = b * T + j * QB
                pa_t = [acc_tile(acc_base + hl) for hl in (0, 1)]
                pa = [t[0:65, :] for t in pa_t]
                nkt = 4 * (j + 1)
                nkp = nkt // 2
                for kp in range(nkp):
                    # prq[p, head, slot, q]: probs for keys 256kp+128*slot+p
                    prq = probs_p.tile([128, 2, 2, 512], F8, tag="pr")
                    d0 = 2 * kp - 4 * j            # diag offset of slot 0
                    c0p = 128 * d0 if d0 > 0 else 0
                    for o in range(2):
                        kt = 2 * kp + o
                        koff = b * T + kt * 128
                        d = kt - 4 * j
                        c0 = 128 * d if d > 0 else 0
                        ps2 = ps_pair.tile([128, 1024], FP32, tag="sc")
                        for hl in (0, 1):
                            hsl = slice(hl * 64, (hl + 1) * 64)
                            nc.tensor.matmul(
                                ps2[:, hl * 512 + c0:(hl + 1) * 512],
                                kt_sb[hsl, koff:koff + 128],
                                qt_sb[hsl, q0 + c0:q0 + QB],
                                start=True, stop=True, skip_group_check=True)
                        # one exp drains both heads' scores -> fp8 probs
                        nc.scalar.activation(
                            out=prq[:, :, o, c0p:],
                            in_=ps2.rearrange("p (h q) -> p h q", h=2)[:, :, c0p:],
                            func=AF.Exp, scale=scale)
                        if d > 0 and c0 > c0p:
                            # below-diagonal cols of the odd slot: zero them
                            nc.vector.memset(prq[:, :, o, c0p:c0], 0.0)
                        if d >= 0:
                            nc.vector.tensor_mul(prq[:, :, o, c0:c0 + 128],
                                                 prq[:, :, o, c0:c0 + 128], tri_sb)
                        yield
                    if b == 1:
                        # HAM warmers for the exp-bound solo-batch tail
                        for _ in range(2):
                            nc.tensor.matmul(
                                pa_t[0][96:128, 64:512], ones_sb[:, 0:32],
                                qt_sb[:, 0:448], start=True, stop=True,
                                skip_group_check=True, tile_position=(0, 96))
                    vtp = vtps[(b * T) // 256 + kp]
                    for hl in (0, 1):
                        nc.tensor.matmul(
                            pa[hl][:, c0p:], vtp[:, :, hl * 65:hl * 65 + 65],
                            prq[:, hl, :, c0p:],
                            start=(kp == 0), stop=(kp == nkp - 1),
                            skip_group_check=True,
                            perf_mode=mybir.MatmulPerfMode.DoubleRow)
                    yield
                for hl in (0, 1):
                    ao = attn_sb_p.tile([65, 512], BF16, tag="ao")
                    nc.vector.tensor_copy(ao, pa[hl])
                    nc.gpsimd.dma_start(
                        out=out_d[hl * 65:(hl + 1) * 65, q0:q0 + QB], in_=ao)
                yield

        # attention gets 3 emission turns per pairs_gen turn: a data-gated
        # rstd/stats op emitted ahead of an exp stalls the whole in-order
        # ACT queue, so keep the exp stream in front.
        gens = [(pairs_gen(), 1), (attn_batch(0), 3), (attn_batch(1), 3)]
        while gens:
            for item in list(gens):
                g, w = item
                try:
                    for _ in range(w):
                        next(g)
                except StopIteration:
                    gens.remove(item)
    nc.compile()
    return nc


FP8 = mybir.dt.float8e4
f8 = ml_dtypes.float8_e4m3


def build_l2():
    """Row-parallel proj+LN2+FFN, transposed end-to-end.

    proj runs in fp8 DoubleRow (atq/wpq pair-interleaved, x16 weight
    pre-scale folded out in the drain); FFN GEMMs stay bf16 (fp8 there
    blows the 2e-2 error budget). PSUM drains ride on DVE so ACT only
    does Ln/Exp/Relu. LN2 stats come from fp8 DR ones-matmuls over
    pair-packed x2. w1 streams (used once); w2 is resident.
    """
    nc = bacc.Bacc("TRN2", target_bir_lowering=False, debug=False, num_devices=NCORES)
    CP = CT // 2
    atq_d = nc.dram_tensor("atq8", [128, CP, 2, ROWS], FP8, kind="ExternalInput").ap()
    wpq_d = nc.dram_tensor("wpq8", [128, CT, CP, 2, 128], FP8, kind="ExternalInput").ap()
    xr_d = nc.dram_tensor("xrt", [128, CT, ROWS], FP32, kind="ExternalInput").ap()
    w1q_d = nc.dram_tensor("w1q", [128, HT, CT * 128], BF16, kind="ExternalInput").ap()
    w2q_d = nc.dram_tensor("w2q", [128, HT, C], BF16, kind="ExternalInput").ap()
    b1_d = nc.dram_tensor("b1r", [128, HT], FP32, kind="ExternalInput").ap()
    b2_d = nc.dram_tensor("b2c", [128, CT], FP32, kind="ExternalInput").ap()
    out_d = nc.dram_tensor("outT", [C, ROWS], FP32, kind="ExternalOutput").ap()

    with tile.TileContext(nc) as tc, contextlib.ExitStack() as ctx:
        consts = ctx.enter_context(tc.tile_pool(name="consts", bufs=1))
        persist = ctx.enter_context(tc.tile_pool(name="persist", bufs=1))
        wstream = ctx.enter_context(tc.tile_pool(name="wstream", bufs=4))
        tmp = ctx.enter_context(tc.tile_pool(name="tmp", bufs=2))
        small = ctx.enter_context(tc.tile_pool(name="small", bufs=1))
        ps_main = ctx.enter_context(tc.tile_pool(name="ps_main", bufs=6, space="PSUM"))
        ps_stats = ctx.enter_context(tc.tile_pool(name="ps_stats", bufs=1, space="PSUM"))

        ones_w = consts.tile([128, 128], BF16)
        nc.vector.memset(ones_w, 1.0)
        ones8 = consts.tile([128, 2, 128], FP8)
        nc.vector.memset(ones8, 1.0)
        dummy_m = consts.tile([128, 512], BF16)
        nc.vector.memset(dummy_m, 0.0)
        eps_sb = consts.tile([128, 1], FP32)
        nc.vector.memset(eps_sb, EPS)

        # PE warm-up spin while the first DMAs land
        warm_ps = ps_main.tile([128, 512], FP32, tag="mm")
        for _ in range(36):
            nc.tensor.matmul(warm_ps[:, 0:128], ones_w, ones_w[:, 0:128],
                             start=True, stop=True)

        # ---- DMAs: spread across engine queues so transfers overlap ----
        # wpq is n8-major, one tile per output chunk: proj for chunk n8
        # starts as soon as its own slice lands. Chunk 0 + atq are issued
        # first: the first stationary load gates proj.
        b1_sb = consts.tile([128, HT], FP32)
        nc.gpsimd.dma_start(out=b1_sb, in_=b1_d)
        b2_sb = consts.tile([128, CT], FP32)
        nc.gpsimd.dma_start(out=b2_sb, in_=b2_d)
        wpq_sb = []
        for n8 in range(CT):
            wpt = consts.tile([128, CP, 2, 128], FP8, tag=f"wpq{n8}", name=f"wpq{n8}")
            wpq_sb.append(wpt)
        nc.scalar.dma_start(out=wpq_sb[0], in_=wpq_d[:, 0])
        atq_sb = consts.tile([128, CP, 2, ROWS], FP8)
        nc.sync.dma_start(out=atq_sb[:, 0:2], in_=atq_d[:, 0:2])
        nc.gpsimd.dma_start(out=atq_sb[:, 2:4], in_=atq_d[:, 2:4])
        for n8 in range(1, CT):
            eng = nc.sync if n8 % 2 == 0 else nc.gpsimd
            eng.dma_start(out=wpq_sb[n8], in_=wpq_d[:, n8])
        xr_sb = consts.tile([128, CT, ROWS], FP32)
        nc.scalar.dma_start(out=xr_sb, in_=xr_d)
        w2_sb = consts.tile([128, HT, C], BF16)
        for g in range(8):
            nc.scalar.dma_start(out=w2_sb[:, g * 4:(g + 1) * 4],
                                in_=w2q_d[:, g * 4:(g + 1) * 4])

        # ---- proj (fp8 DR) + residual -> x2T (fp32); LN2 stats on the fly ----
        x2_sb = persist.tile([128, CT, ROWS], FP32, tag="x2")
        ps_sum = ps_stats.tile([128, 512], FP32, tag="sum")
        ps_sq = ps_stats.tile([128, 512], FP32, tag="sq")
        for n8 in range(CT):
            pp = ps_main.tile([128, 512], FP32, tag="mm")
            for cp in range(CP):
                nc.tensor.matmul(pp, wpq_sb[n8][:, cp],
                                 atq_sb[:, cp], start=(cp == 0), stop=(cp == CP - 1),
                                 skip_group_check=True,
                                 perf_mode=mybir.MatmulPerfMode.DoubleRow)
            # x2 = pp/16 + xr  (undo the x16 wp pre-scale in the drain)
            nc.vector.scalar_tensor_tensor(
                x2_sb[:, n8], pp, 1.0 / 16.0, xr_sb[:, n8],
                op0=mybir.AluOpType.mult, op1=mybir.AluOpType.add)
            if n8 % 2 == 1:
                p = n8 // 2
                x2b8 = tmp.tile([128, 2, 512], FP8, tag="x2b8")
                nc.vector.tensor_copy(x2b8, x2_sb[:, n8 - 1:n8 + 1])
                sq8t = tmp.tile([128, 2, 512], FP8, tag="sq8t")
                nc.gpsimd.tensor_mul(sq8t, x2b8, x2b8)
                nc.tensor.matmul(ps_sum, ones8, x2b8,
                                 start=(p == 0), stop=(p == CP - 1),
                                 skip_group_check=True,
                                 perf_mode=mybir.MatmulPerfMode.DoubleRow)
                nc.tensor.matmul(ps_sq, ones8, sq8t,
                                 start=(p == 0), stop=(p == CP - 1),
                                 skip_group_check=True,
                                 perf_mode=mybir.MatmulPerfMode.DoubleRow)

        # keep the PE HAM window busy through the LN2 dependency stall
        fill_ps = ps_main.tile([128, 512], FP32, tag="mm")
        for _ in range(40):
            nc.tensor.matmul(fill_ps, ones_w, dummy_m, start=True, stop=True,
                             skip_group_check=True)

        # ---- LN2 scalars (broadcast across partitions by construction) ----
        mu = small.tile([128, 512], FP32, tag="mu")
        nc.vector.tensor_scalar_mul(mu, ps_sum, 1.0 / C)
        var = small.tile([128, 512], FP32, tag="var")
        nc.vector.tensor_scalar_mul(var, ps_sq, 1.0 / C)
        mu2 = small.tile([128, 512], FP32, tag="mu2")
        nc.vector.tensor_mul(mu2, mu, mu)
        nc.vector.tensor_sub(var, var, mu2)
        nc.scalar.activation(out=var, in_=var, func=AF.Ln, bias=eps_sb, scale=1.0)
        rstd = small.tile([128, 512], FP32, tag="rstd")
        nc.scalar.activation(out=rstd, in_=var, func=AF.Exp, scale=-0.5)
        murstd = small.tile([128, 512], FP32, tag="murstd")
        nc.vector.tensor_mul(murstd, mu, rstd)

        # ---- h2: (x2 - mu) * rstd, bf16, feature-on-partition ----
        h2_sb = persist.tile([128, CT, ROWS], BF16, tag="h2")
        for ci in range(CT):
            th = tmp.tile([128, 512], FP32, tag="th")
            nc.vector.tensor_mul(th, x2_sb[:, ci], rstd)
            nc.vector.tensor_sub(h2_sb[:, ci], th, murstd)

        # ---- FFN1: h1[ht] = relu(W1g.T @ h2 + b1e); w1 streamed ----
        h1_sb = persist.tile([128, HT, ROWS], BF16, tag="h1")
        for ht in range(HT):
            w1t = wstream.tile([128, CT, 128], BF16, tag="w1t")
            weng = nc.sync if ht % 2 == 0 else nc.gpsimd
            weng.dma_start(out=w1t,
                           in_=w1q_d[:, ht, :].rearrange("p (a m) -> p a m", a=CT))
            ph = ps_main.tile([128, 512], FP32, tag="mm")
            for ci in range(CT):
                nc.tensor.matmul(ph, w1t[:, ci], h2_sb[:, ci],
                                 start=(ci == 0), stop=(ci == CT - 1),
                                 skip_group_check=True)
            nc.scalar.activation(out=h1_sb[:, ht], in_=ph,
                                 func=AF.Relu, bias=b1_sb[:, ht:ht + 1], scale=1.0)

        # ---- FFN2 + residual + b2 -> outT ----
        for pn in range(4):
            pos = []
            for k in range(2):
                po = ps_main.tile([128, 512], FP32, tag="mm")
                pos.append(po)
            for ht in range(HT):
                for k in range(2):
                    n8 = pn * 2 + k
                    nc.tensor.matmul(pos[k], w2_sb[:, ht, n8 * 128:(n8 + 1) * 128],
                                     h1_sb[:, ht], start=(ht == 0), stop=(ht == HT - 1),
                                     skip_group_check=True)
            for k in range(2):
                n8 = pn * 2 + k
                ot = tmp.tile([128, 512], FP32, tag="ot")
                # out = (psum + b2) + x2 in one DVE op
                nc.vector.scalar_tensor_tensor(
                    ot, pos[k], b2_sb[:, n8:n8 + 1], x2_sb[:, n8],
                    op0=mybir.AluOpType.add, op1=mybir.AluOpType.add)
                eng = nc.sync if k == 0 else nc.gpsimd
                eng.dma_start(out=out_d[n8 * 128:(n8 + 1) * 128, :], in_=ot)
    nc.compile()
    return nc


# ---------------- host glue ----------------

def _il8(a):
    """[C, M] -> fp8 pair-interleaved [128, CP, 2, M]."""
    cp = C // 256
    return np.ascontiguousarray(np.clip(a, -240.0, 240.0).astype(f8)
                                .reshape(cp, 2, 128, a.shape[1])
                                .transpose(2, 0, 1, 3))


def prep_l1_inputs(inputs):
    x = np.asarray(inputs["x"], np.float32).reshape(TOK, C)
    g1 = np.asarray(inputs["g1"], np.float32)
    beta1 = np.asarray(inputs["beta1"], np.float32)
    xt = _il8(np.ascontiguousarray(x.T))           # [128, CP, 2, TOK]
    xf = xt.astype(np.float32)
    sq8 = np.ascontiguousarray(
        np.clip(xf * xf, -240.0, 240.0).astype(f8)
        .reshape(128, C // 256, 2, NCH, 512).transpose(3, 0, 1, 2, 4))
    xt = np.ascontiguousarray(
        xt.reshape(128, C // 256, 2, NCH, 512).transpose(3, 0, 1, 2, 4))
    W16 = 16.0
    wq = W16 * g1[:, None] * np.asarray(inputs["Wq"], np.float32)
    wk = W16 * g1[:, None] * np.asarray(inputs["Wk"], np.float32)
    wv = W16 * g1[:, None] * np.asarray(inputs["Wv"], np.float32)
    tri2 = np.broadcast_to(
        np.triu(np.ones((128, 128), np.float32))[:, None, :], (128, 2, 128))
    tri2 = np.ascontiguousarray(tri2.astype(f8))
    in_maps = []
    for c in range(NCORES):
        csl = slice(c * 128, (c + 1) * 128)
        w8 = [_il8(w[:, csl]) for w in (wq, wk, wv)]
        wdq = [w.astype(np.float32).transpose(1, 2, 0, 3).reshape(C, 128)
               for w in w8]
        nws = np.stack([-w.sum(0) for w in wdq], axis=1)
        wb = np.stack([(w / W16).T @ beta1 for w in wdq], axis=1)
        in_maps.append({
            "xt": xt,
            "sq8": sq8,
            "wq": w8[0],
            "wk": w8[1],
            "wv": w8[2],
            "nws": np.ascontiguousarray(nws.astype(np.float32)),
            "wb": np.ascontiguousarray(wb.astype(np.float32)),
            "tri2": tri2,
        })
    return in_maps


def l1_postprocess(r1_results):
    """[130, TOK] per-core (attn_un + denom rows) -> normalized [C, TOK]."""
    blocks = []
    for c in range(NCORES):
        o = np.asarray(r1_results[c]["attn_out"], np.float32).reshape(2, 65, TOK)
        blocks.append(o[:, :64, :] / o[:, 64:65, :])
    return np.concatenate(blocks, axis=0).reshape(C, TOK)


def prep_l2_inputs(inputs, attn_t):
    attn_t = np.asarray(attn_t, np.float32)  # [C, TOK]
    x = np.asarray(inputs["x"], np.float32).reshape(TOK, C)
    x = x + np.asarray(inputs["bp"], np.float32)[None, :]
    g2 = np.asarray(inputs["g2"], np.float32)
    beta2 = np.asarray(inputs["beta2"], np.float32)
    wp = np.asarray(inputs["Wp"], np.float32)
    w1 = np.asarray(inputs["W1"], np.float32)
    w2 = np.asarray(inputs["W2"], np.float32)
    # proj weights: fp8 pair-interleaved, x16 pre-scale (undone in drain)
    wpq = np.ascontiguousarray(np.stack(
        [_il8(16.0 * wp[:, n8 * 128:(n8 + 1) * 128]) for n8 in range(CT)], axis=1))
    w1g = (g2[:, None] * w1).astype(bf16)
    w1q = np.ascontiguousarray(
        w1g.reshape(CT, 128, HT, 128).transpose(1, 2, 0, 3).reshape(128, HT, CT * 128))
    w2q = np.ascontiguousarray(
        w2.astype(bf16).reshape(HT, 128, C).transpose(1, 0, 2))
    b1e = np.asarray(inputs["b1"], np.float32) + w1.T @ beta2
    b1r = np.ascontiguousarray(b1e.reshape(HT, 128).T)
    b2c = np.ascontiguousarray(
        np.asarray(inputs["b2"], np.float32).reshape(CT, 128).T)
    in_maps = []
    for c in range(NCORES):
        rsl = slice(c * ROWS, (c + 1) * ROWS)
        atq8 = _il8(attn_t[:, rsl])
        xrt = np.ascontiguousarray(
            x[rsl, :].T.reshape(CT, 128, ROWS).transpose(1, 0, 2))
        in_maps.append({
            "atq8": atq8,
            "wpq8": wpq,
            "xrt": xrt,
            "w1q": w1q,
            "w2q": w2q,
            "b1r": b1r,
            "b2c": b2c,
        })
    return in_maps


_CACHE = {}


def _get_programs(use_beta):
    key = ("progs", bool(use_beta))
    if key not in _CACHE:
        nc1 = build_l1(use_beta=use_beta)
        nc2 = build_l2()
        _CACHE[key] = (nc1, nc2)
    return _CACHE[key]


def kernel(**inputs):
    from concourse.bass_utils import run_bass_kernel_spmd

    inputs = {k: np.asarray(v) for k, v in inputs.items()}
    use_beta = bool(np.any(np.asarray(inputs["beta1"], np.float32) != 0.0))
    nc1, nc2 = _get_programs(use_beta)
    core_ids = list(range(NCORES))

    r1 = run_bass_kernel_spmd(nc1, prep_l1_inputs(inputs), core_ids)
    attn_t = l1_postprocess(r1.results)

    r2 = run_bass_kernel_spmd(nc2, prep_l2_inputs(inputs, attn_t), core_ids)
    out = np.concatenate(
        [np.asarray(r2.results[c]["outT"]).T for c in range(NCORES)], axis=0)
    return np.ascontiguousarray(out.reshape(B, T, C).astype(np.float32))

